# revision 5
# baseline (speedup 1.0000x reference)
"""Trainium2 Bass kernel for GQA fractal attention (B=2, L=2048, D=1024,
8 heads, 2 query groups, fractal per-key-group scale, masked softmax,
output projection, residual + LayerNorm).

Sharding: 8 cores = batch (2) x query-chunk (4 x 512 rows). Each core
computes full K/V for its batch (duplicated across the 4 cores sharing a
batch), scores/attention for its 512 query rows, then output projection +
residual + LayerNorm for those rows. Host assembles the full output.

All layout gymnastics (transposes, weight permutation, bf16 casts) are done
host-side so the device only ever runs dense contiguous matmuls:
  - activations are stored feature-on-partition (transposed) on chip
  - scores are computed transposed (S^T[k, q]) so the key mask and the
    fractal/softmax scale fold into one scalar-engine Exp activation
  - softmax denominators come from ones-vector matmuls; the reciprocal is
    broadcast with a rank-1 matmul and multiplied into P^T before the PV
    matmul, so no per-attn-row normalization pass is needed
"""

import sys

if "/opt/trn_rl_repo" not in sys.path:
    sys.path.insert(0, "/opt/trn_rl_repo")

import ml_dtypes
import numpy as np

import concourse.bass as bass
import concourse.mybir as mybir
import concourse.tile as tile
from concourse.vector_clock import ScopedClock

# ---------------------------------------------------------------- constants
P = 128
L = 2048
D = 1024
NQ = 512          # query rows per core
HEADS = 8
QG = 2            # query groups
GD = 512          # per-group feature dim (4 heads x 128)
SCALE = (D // HEADS) ** -0.5
FRACTAL = 2.0
LN_EPS = 1e-5
MASK_BIAS = -150.0   # exp(scaled_score + MASK_BIAS) underflows to exactly 0.0
BF16 = mybir.dt.bfloat16
F32 = mybir.dt.float32
NPBF16 = ml_dtypes.bfloat16

DC = D // P        # 8 feature chunks of 128
LC = L // P        # 16 key chunks of 128
LS = L // 512      # 4 key chunks of 512
QS = NQ // P       # 4 query chunks of 128


def _patch_tile_drain():
    """The public neuronxcc walrus build rejects instructions with more than
    one semaphore wait ("Too many sync wait commands"). Tile's kernel-tail
    drain waits on every used proc's final tick, so split it into a chain of
    single-wait drains on the sync engine."""

    def _drain_and_barrier_split(self, tick_clock, wait_clock):
        nc = self.nc
        drain_inst = nc.sync.drain()
        wait_clock.add_sem_waits(
            drain_inst.ins, ScopedClock({None: tick_clock.global_clock})
        )
        si = drain_inst.ins.sync_info
        if si is not None and len(si.on_wait) > 1:
            waits = list(si.on_wait)
            updates = list(si.on_update)
            drain_inst.ins.sync_info = mybir.SyncInfo(
                on_wait=[waits[0]], on_update=updates
            )
            for w in waits[1:]:
                d2 = nc.sync.drain()
                d2.ins.sync_info = mybir.SyncInfo(on_wait=[w], on_update=[])

        nc.all_engine_barrier()
        assert self.sems is not None
        popped = nc._tile_sem_poison_stack.pop()
        assert popped is self._sem_poison
        nc.clear_and_free_semaphores(list(self.sems.allocated().values()))
        nc.all_engine_barrier()

    tile.TileContext._drain_and_barrier = _drain_and_barrier_split


_patch_tile_drain()


def _split_multi_waits(nc):
    """The public neuronxcc walrus build supports only ONE semaphore wait per
    instruction ("Too many sync wait commands"). Tile's sem-assigner can put
    several waits on one instruction; hoist the extras onto same-engine NoOps
    inserted right before it (engines execute in block order, so waiting
    sequentially is equivalent)."""
    k = 0
    for f in nc.m.functions:
        for bb in f.blocks:
            new = []
            changed = False
            for inst in bb.instructions:
                si = inst.sync_info
                if si is not None and len(si.on_wait) > 1:
                    waits = list(si.on_wait)
                    for w in waits[:-1]:
                        nop = mybir.InstNoOp(
                            name=f"wsplit-{k}",
                            engine=inst.engine,
                            ins=[],
                            outs=[],
                            sync_info=mybir.SyncInfo(on_wait=[w], on_update=[]),
                        )
                        new.append(nop)
                        k += 1
                    inst.sync_info = mybir.SyncInfo(
                        on_wait=[waits[-1]], on_update=list(si.on_update)
                    )
                    changed = True
                new.append(inst)
            if changed:
                bb.instructions = new


def build_nc() -> bass.Bass:
    nc = bass.Bass("TRN2")

    xT = nc.dram_tensor("xT", [D, L], BF16, kind="ExternalInput")
    xqT = nc.dram_tensor("xqT", [D, NQ], BF16, kind="ExternalInput")
    xres = nc.dram_tensor("xres", [NQ, D], F32, kind="ExternalInput")
    wkvT = nc.dram_tensor("wkvT", [D, 2 * D], BF16, kind="ExternalInput")
    wqT = nc.dram_tensor("wqT", [D, D], BF16, kind="ExternalInput")
    woT = nc.dram_tensor("woT", [D, D], BF16, kind="ExternalInput")
    maskb = nc.dram_tensor("maskb", [P, LC], F32, kind="ExternalInput")
    lng = nc.dram_tensor("lng", [D], F32, kind="ExternalInput")
    lnb = nc.dram_tensor("lnb", [D], F32, kind="ExternalInput")
    out = nc.dram_tensor("out", [NQ, D], F32, kind="ExternalOutput")

    with (
        tile.TileContext(nc) as tc,
        tc.tile_pool(name="persist", bufs=1) as persist,
        tc.tile_pool(name="psum", bufs=4, space="PSUM") as psum,
        tc.tile_pool(name="psd", bufs=2, space="PSUM") as psd,
    ):
        # ---- persistent tiles (live for the whole kernel)
        kT_sb = persist.tile([P, DC, L], BF16)        # K^T  [feat(G,h,d), key]
        v_sb = persist.tile([P, LC, D], BF16)         # V    [key, feat(G,h,d)]
        qT_sb = persist.tile([P, DC, NQ], BF16)       # Q^T  [feat(g,h,d), q]
        maskb_sb = persist.tile([P, LC], F32)
        ones_sb = persist.tile([P, 1], BF16)
        onesr_sb = persist.tile([1, P], F32)
        lng_sb = persist.tile([P, D], F32)
        lnb_sb = persist.tile([P, D], F32)
        eps_sb = persist.tile([P, 1], F32)

        nc.sync.dma_start(maskb_sb[:], maskb[:])
        nc.vector.memset(ones_sb[:], 1.0)
        nc.vector.memset(onesr_sb[:], 1.0)
        nc.vector.memset(eps_sb[:], LN_EPS)
        lng_bc = bass.AP(tensor=lng, offset=0, ap=[[0, P], [1, D]])
        lnb_bc = bass.AP(tensor=lnb, offset=0, ap=[[0, P], [1, D]])
        nc.sync.dma_start(lng_sb[:], lng_bc)
        nc.sync.dma_start(lnb_sb[:], lnb_bc)

        # ================================================ phase A: projections
        with tc.tile_pool(name="proj", bufs=1) as proj:
            xT_sb = proj.tile([P, DC, L], BF16)
            xqT_sb = proj.tile([P, DC, NQ], BF16)
            wkvT_sb = proj.tile([P, DC, 2 * D], BF16)
            wqT_sb = proj.tile([P, DC, D], BF16)
            for c in range(DC):
                nc.sync.dma_start(xT_sb[:, c, :], xT[c * P:(c + 1) * P, :])
                nc.sync.dma_start(xqT_sb[:, c, :], xqT[c * P:(c + 1) * P, :])
                nc.sync.dma_start(wkvT_sb[:, c, :], wkvT[c * P:(c + 1) * P, :])
                nc.sync.dma_start(wqT_sb[:, c, :], wqT[c * P:(c + 1) * P, :])

            # K^T[f, l] for all 8 feature chunks (G-major), 4 key column tiles
            for gc in range(DC):
                for ks in range(LS):
                    ps = psum.tile([P, 512], F32, tag="mm")
                    for c in range(DC):
                        nc.tensor.matmul(
                            ps[:],
                            lhsT=wkvT_sb[:, c, gc * P:(gc + 1) * P],
                            rhs=xT_sb[:, c, ks * 512:(ks + 1) * 512],
                            start=(c == 0),
                            stop=(c == DC - 1),
                        )
                    nc.vector.tensor_copy(
                        out=kT_sb[:, gc, ks * 512:(ks + 1) * 512], in_=ps[:]
                    )

            # V[l, f] -- 16 key-row tiles x 2 groups of 512 features
            for G in range(QG):
                for ls in range(LC):
                    ps = psum.tile([P, 512], F32, tag="mm")
                    for c in range(DC):
                        nc.tensor.matmul(
                            ps[:],
                            lhsT=xT_sb[:, c, ls * P:(ls + 1) * P],
                            rhs=wkvT_sb[:, c, D + G * GD:D + (G + 1) * GD],
                            start=(c == 0),
                            stop=(c == DC - 1),
                        )
                    nc.vector.tensor_copy(
                        out=v_sb[:, ls, G * GD:(G + 1) * GD], in_=ps[:]
                    )

            # Q^T[f, q] for this core's 512 query rows
            for fc in range(DC):
                ps = psum.tile([P, 512], F32, tag="mm")
                for c in range(DC):
                    nc.tensor.matmul(
                        ps[:],
                        lhsT=wqT_sb[:, c, fc * P:(fc + 1) * P],
                        rhs=xqT_sb[:, c, :],
                        start=(c == 0),
                        stop=(c == DC - 1),
                    )
                nc.vector.tensor_copy(out=qT_sb[:, fc, :], in_=ps[:])

        # ============================================= phase B/C: attention
        with (
            tc.tile_pool(name="attn", bufs=1) as attn,
            tc.tile_pool(name="late", bufs=1) as late,
            tc.tile_pool(name="zp", bufs=2) as zp,
            tc.tile_pool(name="small", bufs=4) as small,
            tc.tile_pool(name="psb", bufs=2, space="PSUM") as psb,
        ):
            woT_sb = late.tile([P, DC, D], BF16)
            xres_sb = late.tile([P, QS, D], F32)
            outT_sb = late.tile([P, DC, NQ], BF16)
            for c in range(DC):
                nc.sync.dma_start(woT_sb[:, c, :], woT[c * P:(c + 1) * P, :])
            for qs in range(QS):
                nc.sync.dma_start(xres_sb[:, qs, :], xres[qs * P:(qs + 1) * P, :])

            for g in range(QG):
                pt_sb = attn.tile([P, QG, LC, 512], BF16, tag="pt")

                # scores^T + fused scale/mask/exp, per key group G
                for G in range(QG):
                    for ks in range(LC):
                        ps = psum.tile([P, 512], F32, tag="mm")
                        for fc in range(4):
                            nc.tensor.matmul(
                                ps[:],
                                lhsT=kT_sb[:, G * 4 + fc, ks * P:(ks + 1) * P],
                                rhs=qT_sb[:, g * 4 + fc, :],
                                start=(fc == 0),
                                stop=(fc == 3),
                            )
                        nc.scalar.activation(
                            out=pt_sb[:, G, ks, :],
                            in_=ps[:],
                            func=mybir.ActivationFunctionType.Exp,
                            bias=maskb_sb[:, ks:ks + 1],
                            scale=SCALE * (FRACTAL ** G),
                        )

                # softmax denominators (per G), reciprocal, broadcast,
                # then normalize P^T in place
                for G in range(QG):
                    ps_d = psd.tile([1, 512], F32, tag="den")
                    for ks in range(LC):
                        nc.tensor.matmul(
                            ps_d[:],
                            lhsT=ones_sb[:, 0:1],
                            rhs=pt_sb[:, G, ks, :],
                            start=(ks == 0),
                            stop=(ks == LC - 1),
                        )
                    rd_sb = small.tile([1, 512], F32, tag="rd")
                    nc.vector.reciprocal(out=rd_sb[:], in_=ps_d[:])
                    ps_b = psb.tile([P, 512], F32, tag="bc")
                    nc.tensor.matmul(
                        ps_b[:], lhsT=onesr_sb[:], rhs=rd_sb[:],
                        start=True, stop=True,
                    )
                    rbc_sb = small.tile([P, 512], F32, tag="rbc")
                    nc.vector.tensor_copy(out=rbc_sb[:], in_=ps_b[:])
                    for ks in range(LC):
                        nc.vector.tensor_tensor(
                            out=pt_sb[:, G, ks, :],
                            in0=pt_sb[:, G, ks, :],
                            in1=rbc_sb[:],
                            op=mybir.AluOpType.mult,
                        )

                # PV: out^T[d, q] accumulated over both key groups
                for ds in range(4):
                    ps_u = psum.tile([P, 512], F32, tag="mm")
                    n_acc = QG * LC
                    i = 0
                    for G in range(QG):
                        for ks in range(LC):
                            nc.tensor.matmul(
                                ps_u[:],
                                lhsT=v_sb[:, ks, G * GD + ds * P:G * GD + (ds + 1) * P],
                                rhs=pt_sb[:, G, ks, :],
                                start=(i == 0),
                                stop=(i == n_acc - 1),
                            )
                            i += 1
                    nc.vector.tensor_copy(out=outT_sb[:, g * 4 + ds, :], in_=ps_u[:])

            # ======================================== phase D: O-proj + LN
            for qs in range(QS):
                z_sb = zp.tile([P, D], F32, tag="z")
                for js in range(2):
                    ps_y = psum.tile([P, 512], F32, tag="mm")
                    for dc in range(DC):
                        nc.tensor.matmul(
                            ps_y[:],
                            lhsT=outT_sb[:, dc, qs * P:(qs + 1) * P],
                            rhs=woT_sb[:, dc, js * 512:(js + 1) * 512],
                            start=(dc == 0),
                            stop=(dc == DC - 1),
                        )
                    nc.vector.tensor_add(
                        out=z_sb[:, js * 512:(js + 1) * 512],
                        in0=ps_y[:],
                        in1=xres_sb[:, qs, js * 512:(js + 1) * 512],
                    )
                # LayerNorm over the 1024 free-axis elements
                stats = small.tile([P, 2, 6], F32, tag="stats")
                mv = small.tile([P, 2], F32, tag="mv")
                for h in range(2):
                    nc.vector.bn_stats(
                        out=stats[:, h, :], in_=z_sb[:, h * 512:(h + 1) * 512]
                    )
                nc.vector.bn_aggr(out=mv[:], in_=stats[:])
                rstd = small.tile([P, 1], F32, tag="rstd")
                nc.scalar.activation(
                    out=rstd[:],
                    in_=mv[:, 1:2],
                    func=mybir.ActivationFunctionType.Sqrt,
                    bias=eps_sb[:],
                    scale=1.0,
                )
                nc.vector.reciprocal(out=rstd[:], in_=rstd[:])
                nc.vector.tensor_scalar(
                    out=z_sb[:],
                    in0=z_sb[:],
                    scalar1=mv[:, 0:1],
                    scalar2=rstd[:],
                    op0=mybir.AluOpType.subtract,
                    op1=mybir.AluOpType.mult,
                )
                nc.vector.tensor_tensor(
                    out=z_sb[:], in0=z_sb[:], in1=lng_sb[:],
                    op=mybir.AluOpType.mult,
                )
                nc.vector.tensor_tensor(
                    out=z_sb[:], in0=z_sb[:], in1=lnb_sb[:],
                    op=mybir.AluOpType.add,
                )
                nc.sync.dma_start(out[qs * P:(qs + 1) * P, :], z_sb[:])

    _split_multi_waits(nc)
    return nc


def make_in_maps(x, mask, Wq, Wkv, Wo, ln_g, ln_b):
    """Host-side prep: per-core transposed/bf16/pre-permuted input arrays."""
    x = np.asarray(x, np.float32)
    mask = np.asarray(mask)
    Wq = np.asarray(Wq, np.float32)
    Wkv = np.asarray(Wkv, np.float32)
    Wo = np.asarray(Wo, np.float32)
    ln_g = np.asarray(ln_g, np.float32)
    ln_b = np.asarray(ln_b, np.float32)

    # Permute Wkv rows so K features (G-major: G, h, d) come first, then V.
    A = Wkv.reshape(HEADS, 2, P, D)
    kw = A[:, 0].reshape(QG, 4, P, D).reshape(D, D)
    vw = A[:, 1].reshape(QG, 4, P, D).reshape(D, D)
    wkvT = np.concatenate([kw, vw], axis=0).T.astype(NPBF16).copy()  # [D, 2D]
    wqT = Wq.T.astype(NPBF16).copy()
    woT = Wo.T.astype(NPBF16).copy()

    in_maps = []
    for core in range(8):
        b, qc = core // 4, core % 4
        q0 = qc * NQ
        xb = x[b]                                   # [L, D]
        xT = xb.T.astype(NPBF16).copy()             # [D, L]
        xqT = xb[q0:q0 + NQ].T.astype(NPBF16).copy()
        xres = xb[q0:q0 + NQ].copy()
        mb = np.where(mask[b], np.float32(MASK_BIAS), np.float32(0.0))
        maskb = mb.reshape(LC, P).T.copy()          # [P, LC]
        in_maps.append({
            "xT": xT, "xqT": xqT, "xres": xres,
            "wkvT": wkvT, "wqT": wqT, "woT": woT,
            "maskb": maskb, "lng": ln_g.copy(), "lnb": ln_b.copy(),
        })
    return in_maps


_NC_CACHE = {}


def get_nc() -> bass.Bass:
    if "nc" not in _NC_CACHE:
        _NC_CACHE["nc"] = build_nc()
    return _NC_CACHE["nc"]


def kernel(**inputs) -> np.ndarray:
    from concourse.bass_utils import run_bass_kernel_spmd

    in_maps = make_in_maps(
        inputs["x"], inputs["mask"], inputs["Wq"], inputs["Wkv"],
        inputs["Wo"], inputs["ln_g"], inputs["ln_b"],
    )
    nc = get_nc()
    res = run_bass_kernel_spmd(nc, in_maps, core_ids=list(range(8)))
    B = 2
    full = np.empty((B, L, D), np.float32)
    for core in range(8):
        b, qc = core // 4, core % 4
        full[b, qc * NQ:(qc + 1) * NQ] = res.results[core]["out"]
    return full
